# revision 7
# baseline (speedup 1.0000x reference)
"""LoRA QKV projection kernel for Trainium2 (Bass/Tile), 8-core SPMD.

Problem: x [B=4, S=2048, D=4096] fp32; for each of q/k/v:
    out = x @ W.T + (x @ A.T) @ B.T      (W [H=4096, D], A [R=16, D], B [H, R])

Sharding: data-parallel over tokens. Each of the 8 cores owns 1024 of the
8192 tokens and computes all 3*4096 output columns for them. Weights are
replicated.

Precision / speed design (measured on hw: 216 ns per 512-wide matmul for
bf16 and for fp8 DoubleRow, 227 ns for f32r; DoubleRow contracts K=256 per
instruction vs 128):
- d-tiles 0..27 (k < 3584) run bf16 x bf16.
- d-tiles 28..31 (k >= 3584) run fp8e4(E4M3) DoubleRow: x scaled by 1/4 and
  W by 4 (product scale exactly 1) accumulate into the same f32 psum. This
  replaces 4 bf16 matmuls with 2 DR matmuls per accumulation group.
- LoRA runs bf16: the three A matrices fuse into one [D, 48] operand
  (xa = x @ A.T in one pass over full D), and each chunk's rank-16 closing
  matmul uses a [48, 512] B operand zero-padded outside its projection's
  rows.
Simulated end-to-end max rel err vs fp64 on the real inputs: 1.42e-2
(tolerance 2e-2); measured bf16-only part contributes ~1.6e-3.

Schedule notes:
- All operands DMA directly into their compute dtypes (no on-chip casts).
- Chunk 0 is split into two 4-token-tile halves so the LoRA prologue (which
  needs the full x) interleaves with chunk 0's first half inside the x-load
  DMA window; with 4+2 psum banks in use the PE never idles waiting on x.
- Output DMAs issue on the Activation (scalar) HWDGE queue so the w stream
  on the sync queue never waits behind 2 MB of outputs per chunk.
- Eviction interleaves per token-tile with the closing matmuls.
"""

import sys
import types

import numpy as np
import ml_dtypes

import concourse.bass as bass
import concourse.mybir as mybir
import concourse.tile as tile
from concourse import bacc, bass_utils


def _install_profiling_shim():
    """Make trace=True usable under axon on images whose ``antenv`` lacks
    ``axon_hooks``: inject the module and register the ctypes NTFF hook.
    Harmless no-op when the real module exists. Also keep profile artifacts
    local (no bucket upload is available here)."""
    try:
        if "antenv.axon_hooks" not in sys.modules:
            try:
                from antenv import axon_hooks  # noqa: F401
            except ImportError:
                mod = types.ModuleType("antenv.axon_hooks")
                mod._hook = None
                mod.set_axon_ntff_profile_hook = lambda h: setattr(
                    mod, "_hook", h)
                mod.get_axon_ntff_profile_hook = lambda: mod._hook
                sys.modules["antenv.axon_hooks"] = mod
                import antenv
                antenv.axon_hooks = mod
                try:
                    from trn_agent_boot.trn_boot import _ntff_profile_via_ctypes
                    hook = _ntff_profile_via_ctypes("/opt/axon/libaxon_pjrt.so")
                    if hook is not None:
                        mod.set_axon_ntff_profile_hook(hook)
                except Exception:
                    pass
        bass_utils.upload_artifacts = lambda tmpdir: "local://" + str(tmpdir)
    except Exception:
        pass


_install_profiling_shim()

F32 = mybir.dt.float32
BF16 = mybir.dt.bfloat16
F8E4 = mybir.dt.float8e4
DR = mybir.MatmulPerfMode.DoubleRow

N_CORES = 8
P = 128          # partition dim
CH = 512         # matmul moving free dim / psum bank width (fp32)
R3 = 48          # 3 stacked rank-16 LoRA blocks
NF8 = 4          # trailing d-tiles computed in fp8 DoubleRow
NDP = NF8 // 2   # DoubleRow tile-pairs (K=256 each)
SX = 0.25        # fp8 x scale; w scale is 1/SX so products are exact


def _build(D, T, H, n_cores=N_CORES):
    DT = D // P             # 32 d-tiles (full contraction)
    DT_BF = DT - NF8        # bf16 d-tiles
    ST = T // P             # 8 token tiles per core
    CH_PER_PROJ = H // CH
    NCHUNK = 3 * CH_PER_PROJ
    SC = T // CH

    assert ST <= 8, "token tiles must fit in the 8 psum banks"

    nc = bacc.Bacc("TRN2", target_bir_lowering=False, debug=False,
                   num_devices=n_cores)

    xT_d = nc.dram_tensor("xT", [D, T], BF16, kind="ExternalInput")
    wT_d = nc.dram_tensor("wT", [DT_BF * P, 3 * H], BF16,
                          kind="ExternalInput")
    x8_d = nc.dram_tensor("x8", [NDP, P, 2, T], F8E4, kind="ExternalInput")
    w8_d = nc.dram_tensor("w8", [NCHUNK, NDP, P, 2, CH], F8E4,
                          kind="ExternalInput")
    a48_d = nc.dram_tensor("a48", [D, R3], BF16, kind="ExternalInput")
    b48_d = nc.dram_tensor("b48", [NCHUNK, R3, CH], BF16,
                           kind="ExternalInput")
    outs_d = [
        nc.dram_tensor(name, [T, H], F32, kind="ExternalOutput")
        for name in ("q", "k", "v")
    ]

    with tile.TileContext(nc) as tc:
        with (
            tc.tile_pool(name="xp", bufs=1) as xp,
            tc.tile_pool(name="wr", bufs=10) as wr,
            tc.tile_pool(name="w8r", bufs=4) as w8r,
            tc.tile_pool(name="br", bufs=3) as br,
            tc.tile_pool(name="psum", bufs=8, space="PSUM") as psum,
            tc.tile_pool(name="outsb", bufs=8) as outsb,
        ):
            # small operands first so they never queue behind x on the queue
            a48 = xp.tile([P, DT, R3], BF16, tag="a48")
            nc.sync.dma_start(
                a48[:], a48_d.rearrange("(dt p) r -> p dt r", p=P))
            x8t = [xp.tile([P, 2, T], F8E4, tag="x8", bufs=NDP,
                           name=f"x8_{dp}") for dp in range(NDP)]
            for dp in range(NDP):
                nc.sync.dma_start(x8t[dp][:], x8_d[dp])

            xt = [xp.tile([P, T], BF16, tag="xt", bufs=DT, name=f"xt_{d}")
                  for d in range(DT)]
            for d in range(DT):
                nc.sync.dma_start(xt[d][:], xT_d[d * P:(d + 1) * P, :])

            xa = xp.tile([R3, T], BF16, tag="xa")

            def dma_w(j, d):
                pj, hoff = j // CH_PER_PROJ, (j % CH_PER_PROJ) * CH
                w = wr.tile([P, CH], BF16, tag="w", name=f"w_{j}_{d}")
                nc.sync.dma_start(
                    w[:],
                    wT_d[d * P:(d + 1) * P,
                         pj * H + hoff:pj * H + hoff + CH],
                )
                return w

            def close_evict(j, ps_tiles, b48, s_base):
                pj, hoff = j // CH_PER_PROJ, (j % CH_PER_PROJ) * CH
                for i, ps in enumerate(ps_tiles):
                    s = s_base + i
                    nc.tensor.matmul(
                        ps[:], xa[:, s * P:(s + 1) * P], b48[:],
                        start=False, stop=True,
                    )
                    ot = outsb.tile([P, CH], F32, tag="o",
                                    name=f"o_{j}_{s}")
                    nc.vector.tensor_copy(ot[:], ps[:])
                    nc.scalar.dma_start(
                        outs_d[pj][s * P:(s + 1) * P, hoff:hoff + CH],
                        ot[:],
                    )

            def fp8_steps(ps_tiles, w8_tiles, s_base):
                for dp in range(NDP):
                    for i, ps in enumerate(ps_tiles):
                        s = s_base + i
                        nc.tensor.matmul(
                            ps[:],
                            x8t[dp][:, :, s * P:(s + 1) * P],
                            w8_tiles[dp][:],
                            start=False, stop=False, perf_mode=DR,
                        )

            # ---- chunk 0 first half (s 0..3) + LoRA prologue, interleaved
            # inside the x-load DMA window (6 psum banks in use) ----
            b48_0 = br.tile([R3, CH], BF16, tag="b", name="b48_0")
            nc.sync.dma_start(b48_0[:], b48_d[0])
            w8_0 = [w8r.tile([P, 2, CH], F8E4, tag="w8", name=f"w8_0_{dp}")
                    for dp in range(NDP)]
            for dp in range(NDP):
                nc.sync.dma_start(w8_0[dp][:], w8_d[0, dp])

            pxa = [psum.tile([R3, CH], F32, tag="ps", name=f"pxa_{sc}")
                   for sc in range(SC)]
            ps0 = [psum.tile([P, CH], F32, tag="ps", name=f"ps_0a_{s}")
                   for s in range(4)]
            for d in range(DT):
                w = dma_w(0, d) if d < DT_BF else None
                for sc in range(SC):
                    nc.tensor.matmul(
                        pxa[sc][:],
                        a48[:, d, :],
                        xt[d][:, sc * CH:(sc + 1) * CH],
                        start=(d == 0),
                        stop=(d == DT - 1),
                    )
                if w is not None:
                    for s in range(4):
                        nc.tensor.matmul(
                            ps0[s][:],
                            xt[d][:, s * P:(s + 1) * P],
                            w[:],
                            start=(d == 0),
                            stop=False,
                        )
            for sc in range(SC):
                nc.vector.tensor_copy(xa[:, sc * CH:(sc + 1) * CH],
                                      pxa[sc][:])
            fp8_steps(ps0, w8_0, 0)
            close_evict(0, ps0, b48_0, 0)

            # ---- chunk 0 second half (s 4..7): w tiles re-fetched ----
            ps0b = [psum.tile([P, CH], F32, tag="ps", name=f"ps_0b_{s}")
                    for s in range(4)]
            for d in range(DT_BF):
                w = dma_w(0, d)
                for i in range(4):
                    s = 4 + i
                    nc.tensor.matmul(
                        ps0b[i][:],
                        xt[d][:, s * P:(s + 1) * P],
                        w[:],
                        start=(d == 0),
                        stop=False,
                    )
            fp8_steps(ps0b, w8_0, 4)
            close_evict(0, ps0b, b48_0, 4)

            # ---- chunks 1..NCHUNK-1: full 8-bank accumulation ----
            for j in range(1, NCHUNK):
                b48 = br.tile([R3, CH], BF16, tag="b", name=f"b48_{j}")
                nc.sync.dma_start(b48[:], b48_d[j])
                w8_j = [w8r.tile([P, 2, CH], F8E4, tag="w8",
                                 name=f"w8_{j}_{dp}") for dp in range(NDP)]
                for dp in range(NDP):
                    nc.sync.dma_start(w8_j[dp][:], w8_d[j, dp])
                ps_tiles = [psum.tile([P, CH], F32, tag="ps",
                                      name=f"ps_{j}_{s}")
                            for s in range(ST)]
                for d in range(DT_BF):
                    w = dma_w(j, d)
                    for s in range(ST):
                        nc.tensor.matmul(
                            ps_tiles[s][:],
                            xt[d][:, s * P:(s + 1) * P],
                            w[:],
                            start=(d == 0),
                            stop=False,
                        )
                fp8_steps(ps_tiles, w8_j, 0)
                close_evict(j, ps_tiles, b48, 0)

    nc.compile()
    return nc


_NC_CACHE = {}


def _get_nc(D, T, H):
    key = (D, T, H)
    if key not in _NC_CACHE:
        _NC_CACHE[key] = _build(D, T, H)
    return _NC_CACHE[key]


def _to_bf16(a):
    """f32 ndarray -> bf16 (round to nearest even), fast bit-twiddle."""
    a = np.ascontiguousarray(a, dtype=np.float32)
    u = a.view(np.uint32)
    rnd = (u >> 16) & 1
    b = ((u + np.uint32(0x7FFF) + rnd) >> 16).astype(np.uint16)
    return b.view(ml_dtypes.bfloat16)


def _run(x, q_weight, k_weight, v_weight, q_A, q_B, k_A, k_B, v_A, v_B,
         trace=False):
    Bb, S, D = x.shape
    H = q_weight.shape[0]
    TOK = Bb * S
    T = TOK // N_CORES
    CH_PER_PROJ = H // CH
    NCHUNK = 3 * CH_PER_PROJ
    DT = D // P
    DT_BF = DT - NF8
    KBF = DT_BF * P          # bf16 contraction size
    E4 = ml_dtypes.float8_e4m3

    nc = _get_nc(D, T, H)

    xf = np.asarray(x, dtype=np.float32).reshape(TOK, D)
    xT = _to_bf16(xf).T                                   # [D, TOK] bf16
    wTf = np.concatenate(
        [np.asarray(w, dtype=np.float32).T
         for w in (q_weight, k_weight, v_weight)], axis=1)  # [D, 3H] f32
    wT = _to_bf16(wTf[:KBF, :])

    # fp8 tail of the contraction: k = KBF + dp*256 + jj*128 + p
    x8f = (xf[:, KBF:].T * np.float32(SX)).astype(E4)     # [NF8*P, TOK]
    x8 = np.ascontiguousarray(
        x8f.reshape(NDP, 2, P, TOK).transpose(0, 2, 1, 3))  # [NDP,P,2,TOK]
    w8f = (wTf[KBF:, :] * np.float32(1.0 / SX)).astype(E4)  # [NF8*P, 3H]
    w8r = w8f.reshape(NDP, 2, P, 3, CH_PER_PROJ, CH)
    w8 = np.ascontiguousarray(
        w8r.transpose(3, 4, 0, 2, 1, 5).reshape(NCHUNK, NDP, P, 2, CH))

    a48 = _to_bf16(
        np.concatenate(
            [np.asarray(a, dtype=np.float32).T for a in (q_A, k_A, v_A)],
            axis=1))
    b48 = np.zeros((NCHUNK, R3, CH), dtype=np.float32)
    for pj, Bm in enumerate((q_B, k_B, v_B)):
        BT = np.asarray(Bm, dtype=np.float32).T          # [16, H]
        for jj in range(CH_PER_PROJ):
            b48[pj * CH_PER_PROJ + jj, 16 * pj:16 * (pj + 1), :] = (
                BT[:, jj * CH:(jj + 1) * CH])
    b48 = _to_bf16(b48)

    in_maps = [
        {"xT": np.ascontiguousarray(xT[:, c * T:(c + 1) * T]),
         "x8": np.ascontiguousarray(x8[:, :, :, c * T:(c + 1) * T]),
         "wT": wT, "w8": w8, "a48": a48, "b48": b48}
        for c in range(N_CORES)
    ]
    res = bass_utils.run_bass_kernel_spmd(
        nc, in_maps, core_ids=list(range(N_CORES)), trace=trace)

    full = []
    for name in ("q", "k", "v"):
        full.append(
            np.concatenate([res.results[c][name] for c in range(N_CORES)],
                           axis=0).reshape(Bb, S, H))
    return tuple(full), res


def kernel(**inputs):
    out, _ = _run(**inputs)
    return out


# revision 8
# speedup vs baseline: 1.0084x; 1.0084x over previous
"""LoRA QKV projection kernel for Trainium2 (Bass/Tile), 8-core SPMD.

Problem: x [B=4, S=2048, D=4096] fp32; for each of q/k/v:
    out = x @ W.T + (x @ A.T) @ B.T      (W [H=4096, D], A [R=16, D], B [H, R])

Sharding: data-parallel over tokens. Each of the 8 cores owns 1024 of the
8192 tokens and computes all 3*4096 output columns for them. Weights are
replicated.

Precision/speed (measured: 216 ns per 512-wide matmul for bf16 and for
fp8 DoubleRow; DoubleRow contracts K=256 per instruction vs 128):
- d-tiles 0..27 (k < 3584) run bf16 x bf16.
- d-tiles 28..31 run fp8 E4M3 DoubleRow: x scaled by 1/4, W by 4 (product
  scale exactly 1) accumulating into the same f32 psum group. Replaces 4
  bf16 matmuls with 2 DR matmuls per group. Measured max rel err 9.2e-3
  (tolerance 2e-2).
- LoRA runs bf16: A matrices fuse into one [D, 128] (zero-padded) operand;
  closing matmuls use [128, 512] B operands zero-padded outside their
  projection's 16 rows, so every matmul is a uniform 128x128 stationary
  (no PE tile-size reconfig stalls).

Schedule notes:
- All operands DMA straight into their compute dtypes (no on-chip casts).
- Chunk 0 is split into two 4-token-tile halves so the LoRA prologue
  (which needs all of x) interleaves with chunk 0's first half inside the
  x-load DMA window; x/w/a48 DMA issue is interleaved so the PE is
  compute-bound within ~10 us of start. Chunk 0's w tiles persist for the
  second half.
- Output DMAs issue on the Activation (scalar) HWDGE queue; psum eviction
  interleaves per token-tile with the closing matmuls.
"""

import sys
import types

import numpy as np
import ml_dtypes

import concourse.bass as bass
import concourse.mybir as mybir
import concourse.tile as tile
from concourse import bacc, bass_utils


def _install_profiling_shim():
    """Make trace=True usable under axon on images whose ``antenv`` lacks
    ``axon_hooks``: inject the module and register the ctypes NTFF hook.
    Harmless no-op when the real module exists. Also keep profile artifacts
    local (no bucket upload is available here)."""
    try:
        if "antenv.axon_hooks" not in sys.modules:
            try:
                from antenv import axon_hooks  # noqa: F401
            except ImportError:
                mod = types.ModuleType("antenv.axon_hooks")
                mod._hook = None
                mod.set_axon_ntff_profile_hook = lambda h: setattr(
                    mod, "_hook", h)
                mod.get_axon_ntff_profile_hook = lambda: mod._hook
                sys.modules["antenv.axon_hooks"] = mod
                import antenv
                antenv.axon_hooks = mod
                try:
                    from trn_agent_boot.trn_boot import _ntff_profile_via_ctypes
                    hook = _ntff_profile_via_ctypes("/opt/axon/libaxon_pjrt.so")
                    if hook is not None:
                        mod.set_axon_ntff_profile_hook(hook)
                except Exception:
                    pass
        bass_utils.upload_artifacts = lambda tmpdir: "local://" + str(tmpdir)
    except Exception:
        pass


_install_profiling_shim()

F32 = mybir.dt.float32
BF16 = mybir.dt.bfloat16
F8E4 = mybir.dt.float8e4
DR = mybir.MatmulPerfMode.DoubleRow

N_CORES = 8
P = 128          # partition dim
CH = 512         # matmul moving free dim / psum bank width (fp32)
R3 = 48          # 3 stacked rank-16 LoRA blocks
RP = 128         # zero-padded operand width so every matmul is 128x128
NF8 = 4          # trailing d-tiles computed in fp8 DoubleRow
NDP = NF8 // 2   # DoubleRow tile-pairs (K=256 each)
SX = 0.25        # fp8 x scale; w scale is 1/SX so products are exact
APARTS = 4       # a48 DMA split for fast startup


def _build(D, T, H, n_cores=N_CORES):
    DT = D // P             # 32 d-tiles (full contraction)
    DT_BF = DT - NF8        # bf16 d-tiles
    ST = T // P             # token tiles per core
    CH_PER_PROJ = H // CH
    NCHUNK = 3 * CH_PER_PROJ
    SC = T // CH

    assert ST <= 8, "token tiles must fit in the 8 psum banks"

    nc = bacc.Bacc("TRN2", target_bir_lowering=False, debug=False,
                   num_devices=n_cores)

    xT_d = nc.dram_tensor("xT", [D, T], BF16, kind="ExternalInput")
    wT_d = nc.dram_tensor("wT", [DT_BF * P, 3 * H], BF16,
                          kind="ExternalInput")
    x8_d = nc.dram_tensor("x8", [NDP, P, 2, T], F8E4, kind="ExternalInput")
    w8_d = nc.dram_tensor("w8", [NCHUNK, NDP, P, 2, CH], F8E4,
                          kind="ExternalInput")
    a48_d = nc.dram_tensor("a48", [D, RP], BF16, kind="ExternalInput")
    b48_d = nc.dram_tensor("b48", [NCHUNK, RP, CH], BF16,
                           kind="ExternalInput")
    outs_d = [
        nc.dram_tensor(name, [T, H], F32, kind="ExternalOutput")
        for name in ("q", "k", "v")
    ]

    with tile.TileContext(nc) as tc:
        with (
            tc.tile_pool(name="xp", bufs=1) as xp,
            tc.tile_pool(name="w0p", bufs=1) as w0p,
            tc.tile_pool(name="wr", bufs=16) as wr,
            tc.tile_pool(name="w8r", bufs=4) as w8r,
            tc.tile_pool(name="br", bufs=3) as br,
            tc.tile_pool(name="psum", bufs=8, space="PSUM") as psum,
            tc.tile_pool(name="outsb", bufs=8) as outsb,
        ):
            a48 = xp.tile([P, DT, RP], BF16, tag="a48")
            a48_src = a48_d.rearrange("(dt p) r -> p dt r", p=P)
            b48_0 = br.tile([RP, CH], BF16, tag="b", name="b48_0")
            x8t = [xp.tile([P, 2, T], F8E4, tag="x8", bufs=NDP,
                           name=f"x8_{dp}") for dp in range(NDP)]
            w8_0 = [w8r.tile([P, 2, CH], F8E4, tag="w8", name=f"w8_0_{dp}")
                    for dp in range(NDP)]

            # x tiles + chunk-0 w tiles + small operands, DMA-issue
            # interleaved so chunk-0 compute starts as soon as possible
            xt = [xp.tile([P, T], BF16, tag="xt", bufs=DT, name=f"xt_{d}")
                  for d in range(DT)]
            w0 = [w0p.tile([P, CH], BF16, tag="w0", bufs=DT_BF,
                           name=f"w0_{d}") for d in range(DT_BF)]
            APD = DT // APARTS
            for d in range(DT):
                nc.sync.dma_start(xt[d][:], xT_d[d * P:(d + 1) * P, :])
                if d < DT_BF:
                    nc.sync.dma_start(w0[d][:],
                                      wT_d[d * P:(d + 1) * P, 0:CH])
                if d < APARTS:
                    nc.sync.dma_start(
                        a48[:, d * APD:(d + 1) * APD, :],
                        a48_src[:, d * APD:(d + 1) * APD, :])
                if d == APARTS:
                    nc.sync.dma_start(b48_0[:], b48_d[0])
                    for dp in range(NDP):
                        nc.sync.dma_start(x8t[dp][:], x8_d[dp])
                        nc.sync.dma_start(w8_0[dp][:], w8_d[0, dp])

            xa = xp.tile([RP, T], BF16, tag="xa")

            def fp8_steps(ps_tiles, w8_tiles, s_base):
                for dp in range(NDP):
                    for i, ps in enumerate(ps_tiles):
                        s = s_base + i
                        nc.tensor.matmul(
                            ps[:],
                            x8t[dp][:, :, s * P:(s + 1) * P],
                            w8_tiles[dp][:],
                            start=False, stop=False, perf_mode=DR,
                        )

            def close_evict(j, ps_tiles, b48, s_base):
                pj, hoff = j // CH_PER_PROJ, (j % CH_PER_PROJ) * CH
                for i, ps in enumerate(ps_tiles):
                    s = s_base + i
                    nc.tensor.matmul(
                        ps[:], xa[:, s * P:(s + 1) * P], b48[:],
                        start=False, stop=True,
                    )
                    ot = outsb.tile([P, CH], F32, tag="o",
                                    name=f"o_{j}_{s}")
                    nc.vector.tensor_copy(ot[:], ps[:])
                    nc.scalar.dma_start(
                        outs_d[pj][s * P:(s + 1) * P, hoff:hoff + CH],
                        ot[:],
                    )

            # ---- chunk 0 first half (s 0..3) + LoRA prologue, interleaved
            # inside the x-load DMA window (6 psum banks in use) ----
            pxa = [psum.tile([RP, CH], F32, tag="ps", name=f"pxa_{sc}")
                   for sc in range(SC)]
            ps0 = [psum.tile([P, CH], F32, tag="ps", name=f"ps_0a_{s}")
                   for s in range(4)]
            for d in range(DT):
                if d < DT_BF:
                    for s in range(4):
                        nc.tensor.matmul(
                            ps0[s][:],
                            xt[d][:, s * P:(s + 1) * P],
                            w0[d][:],
                            start=(d == 0),
                            stop=False,
                        )
                for sc in range(SC):
                    nc.tensor.matmul(
                        pxa[sc][:],
                        a48[:, d, :],
                        xt[d][:, sc * CH:(sc + 1) * CH],
                        start=(d == 0),
                        stop=(d == DT - 1),
                    )
            for sc in range(SC):
                nc.vector.tensor_copy(xa[:, sc * CH:(sc + 1) * CH],
                                      pxa[sc][:])
            fp8_steps(ps0, w8_0, 0)
            close_evict(0, ps0, b48_0, 0)

            # ---- chunk 0 second half (s 4..7): w tiles still resident ----
            ps0b = [psum.tile([P, CH], F32, tag="ps", name=f"ps_0b_{s}")
                    for s in range(4)]
            for d in range(DT_BF):
                for i in range(4):
                    s = 4 + i
                    nc.tensor.matmul(
                        ps0b[i][:],
                        xt[d][:, s * P:(s + 1) * P],
                        w0[d][:],
                        start=(d == 0),
                        stop=False,
                    )
            fp8_steps(ps0b, w8_0, 4)
            close_evict(0, ps0b, b48_0, 4)

            # ---- chunks 1..NCHUNK-1: full 8-bank accumulation ----
            for j in range(1, NCHUNK):
                pj, hoff = j // CH_PER_PROJ, (j % CH_PER_PROJ) * CH
                b48 = br.tile([RP, CH], BF16, tag="b", name=f"b48_{j}")
                nc.sync.dma_start(b48[:], b48_d[j])
                w8_j = [w8r.tile([P, 2, CH], F8E4, tag="w8",
                                 name=f"w8_{j}_{dp}") for dp in range(NDP)]
                for dp in range(NDP):
                    nc.sync.dma_start(w8_j[dp][:], w8_d[j, dp])
                ps_tiles = [psum.tile([P, CH], F32, tag="ps",
                                      name=f"ps_{j}_{s}")
                            for s in range(ST)]
                for d in range(DT_BF):
                    w = wr.tile([P, CH], BF16, tag="w", name=f"w_{j}_{d}")
                    nc.sync.dma_start(
                        w[:],
                        wT_d[d * P:(d + 1) * P,
                             pj * H + hoff:pj * H + hoff + CH],
                    )
                    for s in range(ST):
                        nc.tensor.matmul(
                            ps_tiles[s][:],
                            xt[d][:, s * P:(s + 1) * P],
                            w[:],
                            start=(d == 0),
                            stop=False,
                        )
                fp8_steps(ps_tiles, w8_j, 0)
                close_evict(j, ps_tiles, b48, 0)

    nc.compile()
    return nc


_NC_CACHE = {}


def _get_nc(D, T, H):
    key = (D, T, H)
    if key not in _NC_CACHE:
        _NC_CACHE[key] = _build(D, T, H)
    return _NC_CACHE[key]


def _to_bf16(a):
    """f32 ndarray -> bf16 (round to nearest even), fast bit-twiddle."""
    a = np.ascontiguousarray(a, dtype=np.float32)
    u = a.view(np.uint32)
    rnd = (u >> 16) & 1
    b = ((u + np.uint32(0x7FFF) + rnd) >> 16).astype(np.uint16)
    return b.view(ml_dtypes.bfloat16)


def _run(x, q_weight, k_weight, v_weight, q_A, q_B, k_A, k_B, v_A, v_B,
         trace=False):
    Bb, S, D = x.shape
    H = q_weight.shape[0]
    TOK = Bb * S
    T = TOK // N_CORES
    CH_PER_PROJ = H // CH
    NCHUNK = 3 * CH_PER_PROJ
    DT = D // P
    DT_BF = DT - NF8
    KBF = DT_BF * P
    E4 = ml_dtypes.float8_e4m3

    nc = _get_nc(D, T, H)

    xf = np.asarray(x, dtype=np.float32).reshape(TOK, D)
    xT = _to_bf16(xf).T                                   # [D, TOK] bf16
    wTf = np.concatenate(
        [np.asarray(w, dtype=np.float32).T
         for w in (q_weight, k_weight, v_weight)], axis=1)  # [D, 3H] f32
    wT = _to_bf16(wTf[:KBF, :])

    # fp8 tail of the contraction: k = KBF + dp*256 + jj*128 + p
    x8f = (xf[:, KBF:].T * np.float32(SX)).astype(E4)     # [NF8*P, TOK]
    x8 = np.ascontiguousarray(
        x8f.reshape(NDP, 2, P, TOK).transpose(0, 2, 1, 3))  # [NDP,P,2,TOK]
    w8f = (wTf[KBF:, :] * np.float32(1.0 / SX)).astype(E4)  # [NF8*P, 3H]
    w8r_ = w8f.reshape(NDP, 2, P, 3, CH_PER_PROJ, CH)
    w8 = np.ascontiguousarray(
        w8r_.transpose(3, 4, 0, 2, 1, 5).reshape(NCHUNK, NDP, P, 2, CH))

    a48f = np.zeros((D, RP), dtype=np.float32)
    a48f[:, :R3] = np.concatenate(
        [np.asarray(a, dtype=np.float32).T for a in (q_A, k_A, v_A)], axis=1)
    a48 = _to_bf16(a48f)
    b48 = np.zeros((NCHUNK, RP, CH), dtype=np.float32)
    for pj, Bm in enumerate((q_B, k_B, v_B)):
        BT = np.asarray(Bm, dtype=np.float32).T          # [16, H]
        for jj in range(CH_PER_PROJ):
            b48[pj * CH_PER_PROJ + jj, 16 * pj:16 * (pj + 1), :] = (
                BT[:, jj * CH:(jj + 1) * CH])
    b48 = _to_bf16(b48)

    in_maps = [
        {"xT": np.ascontiguousarray(xT[:, c * T:(c + 1) * T]),
         "x8": np.ascontiguousarray(x8[:, :, :, c * T:(c + 1) * T]),
         "wT": wT, "w8": w8, "a48": a48, "b48": b48}
        for c in range(N_CORES)
    ]
    res = bass_utils.run_bass_kernel_spmd(
        nc, in_maps, core_ids=list(range(N_CORES)), trace=trace)

    full = []
    for name in ("q", "k", "v"):
        full.append(
            np.concatenate([res.results[c][name] for c in range(N_CORES)],
                           axis=0).reshape(Bb, S, H))
    return tuple(full), res


def kernel(**inputs):
    out, _ = _run(**inputs)
    return out


# revision 9
# speedup vs baseline: 1.1371x; 1.1276x over previous
"""LoRA QKV projection kernel for Trainium2 (Bass/Tile), 8-core SPMD.

Problem: x [B=4, S=2048, D=4096] fp32; for each of q/k/v:
    out = x @ W.T + (x @ A.T) @ B.T      (W [H=4096, D], A [R=16, D], B [H, R])

Sharding: data-parallel over tokens. Each of the 8 cores owns 1024 of the
8192 tokens and computes all 3*4096 output columns for them. Weights are
replicated.

On-device math runs the tensor engine in bf16 (both operands): measured
216 ns per 128x512 matmul vs 227 ns for f32r, and bf16 halves SBUF + HBM
traffic. End-to-end max rel err vs fp64 is ~1.6e-3 (tolerance 2e-2).
fp8 DoubleRow was measured to trigger ~20% PE downclocking on sustained
runs, which loses more than its 2x-K-per-instruction saves — not used.

Schedule notes:
- All operands DMA directly into their compute dtypes (no on-chip casts).
- The three LoRA A matrices fuse into one [D, 48] operand so xa = x @ A.T
  is a single pass; each chunk's rank-16 closing matmul uses a [48, 512]
  B operand zero-padded outside its projection's 16 rows.
- Chunk 0 is split into two 4-token-tile halves so the LoRA prologue
  (which needs all of x) interleaves with chunk 0's first half inside the
  x-load DMA window (4+2 psum banks), and x/w DMA issue alternates so the
  PE is compute-bound almost immediately. Chunk 0's w tiles persist in
  SBUF for the second half (no refetch).
- Output DMAs issue on the Activation (scalar) HWDGE queue so the w stream
  on the sync queue never waits behind 2 MB of outputs per chunk; psum
  eviction interleaves per token-tile with the closing matmuls.
"""

import sys
import types

import numpy as np
import ml_dtypes

import concourse.bass as bass
import concourse.mybir as mybir
import concourse.tile as tile
from concourse import bacc, bass_utils


def _install_profiling_shim():
    """Make trace=True usable under axon on images whose ``antenv`` lacks
    ``axon_hooks``: inject the module and register the ctypes NTFF hook.
    Harmless no-op when the real module exists. Also keep profile artifacts
    local (no bucket upload is available here)."""
    try:
        if "antenv.axon_hooks" not in sys.modules:
            try:
                from antenv import axon_hooks  # noqa: F401
            except ImportError:
                mod = types.ModuleType("antenv.axon_hooks")
                mod._hook = None
                mod.set_axon_ntff_profile_hook = lambda h: setattr(
                    mod, "_hook", h)
                mod.get_axon_ntff_profile_hook = lambda: mod._hook
                sys.modules["antenv.axon_hooks"] = mod
                import antenv
                antenv.axon_hooks = mod
                try:
                    from trn_agent_boot.trn_boot import _ntff_profile_via_ctypes
                    hook = _ntff_profile_via_ctypes("/opt/axon/libaxon_pjrt.so")
                    if hook is not None:
                        mod.set_axon_ntff_profile_hook(hook)
                except Exception:
                    pass
        bass_utils.upload_artifacts = lambda tmpdir: "local://" + str(tmpdir)
    except Exception:
        pass


_install_profiling_shim()

F32 = mybir.dt.float32
BF16 = mybir.dt.bfloat16

N_CORES = 8
P = 128          # partition dim
CH = 512         # matmul moving free dim / psum bank width (fp32)
R3 = 48          # 3 stacked rank-16 LoRA blocks
RP = 128         # zero-padded operand width so every matmul is 128x128


def _build(D, T, H, n_cores=N_CORES):
    DT = D // P             # d-tiles
    ST = T // P             # token tiles per core
    CH_PER_PROJ = H // CH
    NCHUNK = 3 * CH_PER_PROJ
    SC = T // CH

    assert ST <= 8, "token tiles must fit in the 8 psum banks"

    nc = bacc.Bacc("TRN2", target_bir_lowering=False, debug=False,
                   num_devices=n_cores)

    xT_d = nc.dram_tensor("xT", [D, T], BF16, kind="ExternalInput")
    wT_d = nc.dram_tensor("wT", [D, 3 * H], BF16, kind="ExternalInput")
    a48_d = nc.dram_tensor("a48", [D, RP], BF16, kind="ExternalInput")
    b48_d = nc.dram_tensor("b48", [NCHUNK, RP, CH], BF16,
                           kind="ExternalInput")
    outs_d = [
        nc.dram_tensor(name, [T, H], F32, kind="ExternalOutput")
        for name in ("q", "k", "v")
    ]

    with tile.TileContext(nc) as tc:
        with (
            tc.tile_pool(name="xp", bufs=1) as xp,
            tc.tile_pool(name="w0p", bufs=1) as w0p,
            tc.tile_pool(name="wr", bufs=16) as wr,
            tc.tile_pool(name="br", bufs=3) as br,
            tc.tile_pool(name="psum", bufs=8, space="PSUM") as psum,
            tc.tile_pool(name="outsb", bufs=8) as outsb,
        ):
            a48 = xp.tile([P, DT, RP], BF16, tag="a48")
            a48_src = a48_d.rearrange("(dt p) r -> p dt r", p=P)
            b48_0 = br.tile([RP, CH], BF16, tag="b", name="b48_0")

            # x tiles, chunk-0 w tiles, and a48 quarters: DMA-issue
            # interleaved so chunk-0 compute starts as soon as possible
            xt = [xp.tile([P, T], BF16, tag="xt", bufs=DT, name=f"xt_{d}")
                  for d in range(DT)]
            w0 = [w0p.tile([P, CH], BF16, tag="w0", bufs=DT,
                           name=f"w0_{d}") for d in range(DT)]
            APARTS = 4
            APD = DT // APARTS
            for d in range(DT):
                nc.sync.dma_start(xt[d][:], xT_d[d * P:(d + 1) * P, :])
                nc.sync.dma_start(w0[d][:], wT_d[d * P:(d + 1) * P, 0:CH])
                if d < APARTS:
                    nc.sync.dma_start(
                        a48[:, d * APD:(d + 1) * APD, :],
                        a48_src[:, d * APD:(d + 1) * APD, :])
                if d == APARTS:
                    nc.sync.dma_start(b48_0[:], b48_d[0])

            xa = xp.tile([RP, T], BF16, tag="xa")

            def close_evict(j, ps_tiles, b48, s_base):
                pj, hoff = j // CH_PER_PROJ, (j % CH_PER_PROJ) * CH
                for i, ps in enumerate(ps_tiles):
                    s = s_base + i
                    nc.tensor.matmul(
                        ps[:], xa[:, s * P:(s + 1) * P], b48[:],
                        start=False, stop=True,
                    )
                    ot = outsb.tile([P, CH], F32, tag="o",
                                    name=f"o_{j}_{s}")
                    nc.vector.tensor_copy(ot[:], ps[:])
                    nc.scalar.dma_start(
                        outs_d[pj][s * P:(s + 1) * P, hoff:hoff + CH],
                        ot[:],
                    )

            # ---- chunk 0 first half (s 0..3) + LoRA prologue, interleaved
            # inside the x-load DMA window (6 psum banks in use) ----
            pxa = [psum.tile([RP, CH], F32, tag="ps", name=f"pxa_{sc}")
                   for sc in range(SC)]
            ps0 = [psum.tile([P, CH], F32, tag="ps", name=f"ps_0a_{s}")
                   for s in range(4)]
            for d in range(DT):
                for s in range(4):
                    nc.tensor.matmul(
                        ps0[s][:],
                        xt[d][:, s * P:(s + 1) * P],
                        w0[d][:],
                        start=(d == 0),
                        stop=False,
                    )
                for sc in range(SC):
                    nc.tensor.matmul(
                        pxa[sc][:],
                        a48[:, d, :],
                        xt[d][:, sc * CH:(sc + 1) * CH],
                        start=(d == 0),
                        stop=(d == DT - 1),
                    )
            for sc in range(SC):
                nc.vector.tensor_copy(xa[:, sc * CH:(sc + 1) * CH],
                                      pxa[sc][:])
            close_evict(0, ps0, b48_0, 0)

            # ---- chunk 0 second half (s 4..7): w tiles still resident ----
            ps0b = [psum.tile([P, CH], F32, tag="ps", name=f"ps_0b_{s}")
                    for s in range(4)]
            for d in range(DT):
                for i in range(4):
                    s = 4 + i
                    nc.tensor.matmul(
                        ps0b[i][:],
                        xt[d][:, s * P:(s + 1) * P],
                        w0[d][:],
                        start=(d == 0),
                        stop=False,
                    )
            close_evict(0, ps0b, b48_0, 4)

            # ---- chunks 1..NCHUNK-1: full 8-bank accumulation ----
            for j in range(1, NCHUNK):
                pj, hoff = j // CH_PER_PROJ, (j % CH_PER_PROJ) * CH
                b48 = br.tile([RP, CH], BF16, tag="b", name=f"b48_{j}")
                nc.sync.dma_start(b48[:], b48_d[j])
                ps_tiles = [psum.tile([P, CH], F32, tag="ps",
                                      name=f"ps_{j}_{s}")
                            for s in range(ST)]
                for d in range(DT):
                    w = wr.tile([P, CH], BF16, tag="w", name=f"w_{j}_{d}")
                    nc.sync.dma_start(
                        w[:],
                        wT_d[d * P:(d + 1) * P,
                             pj * H + hoff:pj * H + hoff + CH],
                    )
                    for s in range(ST):
                        nc.tensor.matmul(
                            ps_tiles[s][:],
                            xt[d][:, s * P:(s + 1) * P],
                            w[:],
                            start=(d == 0),
                            stop=False,
                        )
                close_evict(j, ps_tiles, b48, 0)

    nc.compile()
    return nc


_NC_CACHE = {}


def _get_nc(D, T, H):
    key = (D, T, H)
    if key not in _NC_CACHE:
        _NC_CACHE[key] = _build(D, T, H)
    return _NC_CACHE[key]


def _to_bf16(a):
    """f32 ndarray -> bf16 (round to nearest even), fast bit-twiddle."""
    a = np.ascontiguousarray(a, dtype=np.float32)
    u = a.view(np.uint32)
    rnd = (u >> 16) & 1
    b = ((u + np.uint32(0x7FFF) + rnd) >> 16).astype(np.uint16)
    return b.view(ml_dtypes.bfloat16)


def _run(x, q_weight, k_weight, v_weight, q_A, q_B, k_A, k_B, v_A, v_B,
         trace=False):
    Bb, S, D = x.shape
    H = q_weight.shape[0]
    TOK = Bb * S
    T = TOK // N_CORES
    CH_PER_PROJ = H // CH
    NCHUNK = 3 * CH_PER_PROJ

    nc = _get_nc(D, T, H)

    xT = _to_bf16(np.asarray(x, dtype=np.float32).reshape(TOK, D)).T
    wT = _to_bf16(
        np.concatenate(
            [np.asarray(w, dtype=np.float32).T
             for w in (q_weight, k_weight, v_weight)], axis=1))
    a48f = np.zeros((D, RP), dtype=np.float32)
    a48f[:, :R3] = np.concatenate(
        [np.asarray(a, dtype=np.float32).T for a in (q_A, k_A, v_A)], axis=1)
    a48 = _to_bf16(a48f)
    b48 = np.zeros((NCHUNK, RP, CH), dtype=np.float32)
    for pj, Bm in enumerate((q_B, k_B, v_B)):
        BT = np.asarray(Bm, dtype=np.float32).T          # [16, H]
        for jj in range(CH_PER_PROJ):
            b48[pj * CH_PER_PROJ + jj, 16 * pj:16 * (pj + 1), :] = (
                BT[:, jj * CH:(jj + 1) * CH])
    b48 = _to_bf16(b48)

    in_maps = [
        {"xT": np.ascontiguousarray(xT[:, c * T:(c + 1) * T]),
         "wT": wT, "a48": a48, "b48": b48}
        for c in range(N_CORES)
    ]
    res = bass_utils.run_bass_kernel_spmd(
        nc, in_maps, core_ids=list(range(N_CORES)), trace=trace)

    full = []
    for name in ("q", "k", "v"):
        full.append(
            np.concatenate([res.results[c][name] for c in range(N_CORES)],
                           axis=0).reshape(Bb, S, H))
    return tuple(full), res


def kernel(**inputs):
    out, _ = _run(**inputs)
    return out


# revision 10
# speedup vs baseline: 1.1392x; 1.0019x over previous
"""LoRA QKV projection kernel for Trainium2 (Bass/Tile), 8-core SPMD.

Problem: x [B=4, S=2048, D=4096] fp32; for each of q/k/v:
    out = x @ W.T + (x @ A.T) @ B.T      (W [H=4096, D], A [R=16, D], B [H, R])

Sharding: data-parallel over tokens. Each of the 8 cores owns 1024 of the
8192 tokens and computes all 3*4096 output columns for them. Weights are
replicated.

On-device math runs the tensor engine in bf16 (both operands): measured
216 ns per 128x512 matmul vs 227 ns for f32r, and bf16 halves SBUF + HBM
traffic. End-to-end max rel err vs fp64 is ~1.6e-3 (tolerance 2e-2).
fp8 DoubleRow was measured to trigger ~20% PE downclocking on sustained
runs, which loses more than its 2x-K-per-instruction saves — not used.

Schedule notes:
- All operands DMA directly into their compute dtypes (no on-chip casts).
- The three LoRA A matrices fuse into one [D, 48] operand so xa = x @ A.T
  is a single pass; each chunk's rank-16 closing matmul uses a [48, 512]
  B operand zero-padded outside its projection's 16 rows.
- Chunk 0 is split into two 4-token-tile halves so the LoRA prologue
  (which needs all of x) interleaves with chunk 0's first half inside the
  x-load DMA window (4+2 psum banks), and x/w DMA issue alternates so the
  PE is compute-bound almost immediately. Chunk 0's w tiles persist in
  SBUF for the second half (no refetch).
- Output DMAs issue on the Activation (scalar) HWDGE queue so the w stream
  on the sync queue never waits behind 2 MB of outputs per chunk; psum
  eviction interleaves per token-tile with the closing matmuls.
"""

import sys
import types

import numpy as np
import ml_dtypes

import concourse.bass as bass
import concourse.mybir as mybir
import concourse.tile as tile
from concourse import bacc, bass_utils


def _install_profiling_shim():
    """Make trace=True usable under axon on images whose ``antenv`` lacks
    ``axon_hooks``: inject the module and register the ctypes NTFF hook.
    Harmless no-op when the real module exists. Also keep profile artifacts
    local (no bucket upload is available here)."""
    try:
        if "antenv.axon_hooks" not in sys.modules:
            try:
                from antenv import axon_hooks  # noqa: F401
            except ImportError:
                mod = types.ModuleType("antenv.axon_hooks")
                mod._hook = None
                mod.set_axon_ntff_profile_hook = lambda h: setattr(
                    mod, "_hook", h)
                mod.get_axon_ntff_profile_hook = lambda: mod._hook
                sys.modules["antenv.axon_hooks"] = mod
                import antenv
                antenv.axon_hooks = mod
                try:
                    from trn_agent_boot.trn_boot import _ntff_profile_via_ctypes
                    hook = _ntff_profile_via_ctypes("/opt/axon/libaxon_pjrt.so")
                    if hook is not None:
                        mod.set_axon_ntff_profile_hook(hook)
                except Exception:
                    pass
        bass_utils.upload_artifacts = lambda tmpdir: "local://" + str(tmpdir)
    except Exception:
        pass


_install_profiling_shim()

F32 = mybir.dt.float32
BF16 = mybir.dt.bfloat16

N_CORES = 8
P = 128          # partition dim
CH = 512         # matmul moving free dim / psum bank width (fp32)
R3 = 48          # 3 stacked rank-16 LoRA blocks
RP = 128         # zero-padded operand width so every matmul is 128x128


def _build(D, T, H, n_cores=N_CORES):
    DT = D // P             # d-tiles
    ST = T // P             # token tiles per core
    CH_PER_PROJ = H // CH
    NCHUNK = 3 * CH_PER_PROJ
    SC = T // CH

    assert ST <= 8, "token tiles must fit in the 8 psum banks"

    nc = bacc.Bacc("TRN2", target_bir_lowering=False, debug=False,
                   num_devices=n_cores)

    xT_d = nc.dram_tensor("xT", [D, T], BF16, kind="ExternalInput")
    wT_d = nc.dram_tensor("wT", [D, 3 * H], BF16, kind="ExternalInput")
    a48_d = nc.dram_tensor("a48", [D, RP], BF16, kind="ExternalInput")
    b48_d = nc.dram_tensor("b48", [NCHUNK, RP, CH], BF16,
                           kind="ExternalInput")
    outs_d = [
        nc.dram_tensor(name, [T, H], F32, kind="ExternalOutput")
        for name in ("q", "k", "v")
    ]

    with tile.TileContext(nc) as tc:
        with (
            tc.tile_pool(name="xp", bufs=1) as xp,
            tc.tile_pool(name="w0p", bufs=1) as w0p,
            tc.tile_pool(name="wr", bufs=16) as wr,
            tc.tile_pool(name="br", bufs=3) as br,
            tc.tile_pool(name="psum", bufs=8, space="PSUM") as psum,
            tc.tile_pool(name="outsb", bufs=8) as outsb,
        ):
            a48 = xp.tile([P, DT, RP], BF16, tag="a48")
            a48_src = a48_d.rearrange("(dt p) r -> p dt r", p=P)
            b48_0 = br.tile([RP, CH], BF16, tag="b", name="b48_0")

            # x tiles, chunk-0 w tiles, and a48 quarters: DMA-issue
            # interleaved so chunk-0 compute starts as soon as possible
            xt = [xp.tile([P, T], BF16, tag="xt", bufs=DT, name=f"xt_{d}")
                  for d in range(DT)]
            w0 = [w0p.tile([P, CH], BF16, tag="w0", bufs=DT,
                           name=f"w0_{d}") for d in range(DT)]
            APARTS = 4
            APD = DT // APARTS
            for d in range(DT):
                nc.sync.dma_start(xt[d][:], xT_d[d * P:(d + 1) * P, :])
                nc.sync.dma_start(w0[d][:], wT_d[d * P:(d + 1) * P, 0:CH])
                if d < APARTS:
                    nc.sync.dma_start(
                        a48[:, d * APD:(d + 1) * APD, :],
                        a48_src[:, d * APD:(d + 1) * APD, :])
                if d == APARTS:
                    nc.sync.dma_start(b48_0[:], b48_d[0])

            xa = xp.tile([RP, T], BF16, tag="xa")

            def close_evict(j, ps_tiles, b48, s_base):
                pj, hoff = j // CH_PER_PROJ, (j % CH_PER_PROJ) * CH
                for i, ps in enumerate(ps_tiles):
                    s = s_base + i
                    nc.tensor.matmul(
                        ps[:], xa[:, s * P:(s + 1) * P], b48[:],
                        start=False, stop=True,
                    )
                    ot = outsb.tile([P, CH], F32, tag="o",
                                    name=f"o_{j}_{s}")
                    nc.vector.tensor_copy(ot[:], ps[:])
                    nc.scalar.dma_start(
                        outs_d[pj][s * P:(s + 1) * P, hoff:hoff + CH],
                        ot[:],
                    )

            # ---- chunk 0 first half (s 0..3) + LoRA prologue, interleaved
            # inside the x-load DMA window (6 psum banks in use) ----
            pxa = [psum.tile([RP, CH], F32, tag="ps", name=f"pxa_{sc}")
                   for sc in range(SC)]
            ps0 = [psum.tile([P, CH], F32, tag="ps", name=f"ps_0a_{s}")
                   for s in range(5)]
            for d in range(DT):
                for s in range(5):
                    nc.tensor.matmul(
                        ps0[s][:],
                        xt[d][:, s * P:(s + 1) * P],
                        w0[d][:],
                        start=(d == 0),
                        stop=False,
                    )
                for sc in range(SC):
                    nc.tensor.matmul(
                        pxa[sc][:],
                        a48[:, d, :],
                        xt[d][:, sc * CH:(sc + 1) * CH],
                        start=(d == 0),
                        stop=(d == DT - 1),
                    )
            for sc in range(SC):
                nc.vector.tensor_copy(xa[:, sc * CH:(sc + 1) * CH],
                                      pxa[sc][:])
            close_evict(0, ps0, b48_0, 0)

            # ---- chunk 0 second half (s 4..7): w tiles still resident ----
            ps0b = [psum.tile([P, CH], F32, tag="ps", name=f"ps_0b_{s}")
                    for s in range(3)]
            for d in range(DT):
                for i in range(3):
                    s = 5 + i
                    nc.tensor.matmul(
                        ps0b[i][:],
                        xt[d][:, s * P:(s + 1) * P],
                        w0[d][:],
                        start=(d == 0),
                        stop=False,
                    )
            close_evict(0, ps0b, b48_0, 5)

            # ---- chunks 1..NCHUNK-1: full 8-bank accumulation ----
            for j in range(1, NCHUNK):
                pj, hoff = j // CH_PER_PROJ, (j % CH_PER_PROJ) * CH
                b48 = br.tile([RP, CH], BF16, tag="b", name=f"b48_{j}")
                nc.sync.dma_start(b48[:], b48_d[j])
                ps_tiles = [psum.tile([P, CH], F32, tag="ps",
                                      name=f"ps_{j}_{s}")
                            for s in range(ST)]
                for d in range(DT):
                    w = wr.tile([P, CH], BF16, tag="w", name=f"w_{j}_{d}")
                    nc.sync.dma_start(
                        w[:],
                        wT_d[d * P:(d + 1) * P,
                             pj * H + hoff:pj * H + hoff + CH],
                    )
                    for s in range(ST):
                        nc.tensor.matmul(
                            ps_tiles[s][:],
                            xt[d][:, s * P:(s + 1) * P],
                            w[:],
                            start=(d == 0),
                            stop=False,
                        )
                close_evict(j, ps_tiles, b48, 0)

    nc.compile()
    return nc


_NC_CACHE = {}


def _get_nc(D, T, H):
    key = (D, T, H)
    if key not in _NC_CACHE:
        _NC_CACHE[key] = _build(D, T, H)
    return _NC_CACHE[key]


def _to_bf16(a):
    """f32 ndarray -> bf16 (round to nearest even), fast bit-twiddle."""
    a = np.ascontiguousarray(a, dtype=np.float32)
    u = a.view(np.uint32)
    rnd = (u >> 16) & 1
    b = ((u + np.uint32(0x7FFF) + rnd) >> 16).astype(np.uint16)
    return b.view(ml_dtypes.bfloat16)


def _run(x, q_weight, k_weight, v_weight, q_A, q_B, k_A, k_B, v_A, v_B,
         trace=False):
    Bb, S, D = x.shape
    H = q_weight.shape[0]
    TOK = Bb * S
    T = TOK // N_CORES
    CH_PER_PROJ = H // CH
    NCHUNK = 3 * CH_PER_PROJ

    nc = _get_nc(D, T, H)

    xT = _to_bf16(np.asarray(x, dtype=np.float32).reshape(TOK, D)).T
    wT = _to_bf16(
        np.concatenate(
            [np.asarray(w, dtype=np.float32).T
             for w in (q_weight, k_weight, v_weight)], axis=1))
    a48f = np.zeros((D, RP), dtype=np.float32)
    a48f[:, :R3] = np.concatenate(
        [np.asarray(a, dtype=np.float32).T for a in (q_A, k_A, v_A)], axis=1)
    a48 = _to_bf16(a48f)
    b48 = np.zeros((NCHUNK, RP, CH), dtype=np.float32)
    for pj, Bm in enumerate((q_B, k_B, v_B)):
        BT = np.asarray(Bm, dtype=np.float32).T          # [16, H]
        for jj in range(CH_PER_PROJ):
            b48[pj * CH_PER_PROJ + jj, 16 * pj:16 * (pj + 1), :] = (
                BT[:, jj * CH:(jj + 1) * CH])
    b48 = _to_bf16(b48)

    in_maps = [
        {"xT": np.ascontiguousarray(xT[:, c * T:(c + 1) * T]),
         "wT": wT, "a48": a48, "b48": b48}
        for c in range(N_CORES)
    ]
    res = bass_utils.run_bass_kernel_spmd(
        nc, in_maps, core_ids=list(range(N_CORES)), trace=trace)

    full = []
    for name in ("q", "k", "v"):
        full.append(
            np.concatenate([res.results[c][name] for c in range(N_CORES)],
                           axis=0).reshape(Bb, S, H))
    return tuple(full), res


def kernel(**inputs):
    out, _ = _run(**inputs)
    return out
